# revision 18
# baseline (speedup 1.0000x reference)
"""Chamfer distance + F1 kernel for Trainium2 (8 NeuronCores).

Strategy (B=4 batches, N=M=8192 points, 3D):
  - core c handles batch b = c//2, row-half h = c%2 of xyz1 (4096 rows).
  - PE computes the scaled squared-distance block 4096*d[p,f] in ONE fp16
    matmul pass using an augmented K=13 contraction built on the host:
    each fp32 operand is split into fp16 hi+lo parts so the result is
    accurate to ~1e-6 while streaming at full bf16/fp16 PE rate.
  - ACT converts PSUM fp32 -> SBUF fp16 (with Relu), enabling DVE 2x mode.
  - DVE computes per-row mins (dist1) via a fused custom DVE op
    (out=min(lo,hi), accum_out=min-fold), and a running elementwise min
    across row-tiles (col-min accumulator M, fp16 tensor_tensor at 2x).
  - PE transposes M, DVE reduces -> per-column partial mins (dist2 half).
  - Host combines the two halves per batch and computes cd_p/cd_t/f1 on
    the 8192-element min vectors (0.01% of the FLOPs).
"""

import sys

if "/opt/trn_rl_repo" not in sys.path:
    sys.path.insert(0, "/opt/trn_rl_repo")

from contextlib import ExitStack

import numpy as np

import concourse.tile as tile
import concourse.dve_ops as dve_ops
from concourse import bacc, mybir
from concourse.bass_utils import run_bass_kernel_spmd
from concourse.dve_spec import C0, AluOp, Spec, Src0, Src1, lower, minn
from concourse.dve_uop import DveOpSpec

F16 = mybir.dt.float16
F32 = mybir.dt.float32
MIN = mybir.AluOpType.min
AXX = mybir.AxisListType.X

SCALE = 24.0  # coordinate prescale; distances come out scaled by SCALE**2
# (max pairwise sq-dist for these inputs is ~92; 92*24^2 = 53k < fp16 max)
DSCALE = SCALE * SCALE
F1_THRESHOLD = 1e-4

N_CORES = 8
K_AUG = 13  # 9 coord-product rows + 2 sq1 rows + 2 sq2 rows


def _split16(v):
    """Split fp32 array into fp16 hi + lo so hi+lo ~= v to ~2^-22 rel."""
    hi = v.astype(np.float16)
    lo = (v - hi.astype(np.float32)).astype(np.float16)
    return hi, lo


def _prep_core(xyz1_half, xyz2_full):
    """Build the augmented fp16 operands for one core.

    Returns lhsT [13, n_rows] (stationary, xyz1 side) and
    rhs [13, n_cols] (moving, xyz2 side) such that
    sum_k lhsT[k,p] * rhs[k,f] ~= DSCALE * ||xyz1[p] - xyz2[f]||^2.
    """
    n_rows = xyz1_half.shape[0]
    n_cols = xyz2_full.shape[0]
    v1 = (-2.0 * SCALE) * xyz1_half.astype(np.float32)  # [n_rows, 3]
    w2 = SCALE * xyz2_full.astype(np.float32)  # [n_cols, 3]
    h1, l1 = _split16(v1)
    h2, l2 = _split16(w2)

    s1 = (SCALE * xyz1_half.astype(np.float32)) ** 2
    s1q = s1.sum(axis=1) * 0.25  # DSCALE*sq1 / 4
    s2q = ((w2.astype(np.float64) ** 2).sum(axis=1) * 0.25).astype(np.float32)
    s1h, s1l = _split16(s1q)
    s2h, s2l = _split16(s2q)

    lhsT = np.empty((K_AUG, n_rows), np.float16)
    rhs = np.empty((K_AUG, n_cols), np.float16)
    for c in range(3):
        lhsT[3 * c + 0] = h1[:, c]
        lhsT[3 * c + 1] = h1[:, c]
        lhsT[3 * c + 2] = l1[:, c]
        rhs[3 * c + 0] = h2[:, c]
        rhs[3 * c + 1] = l2[:, c]
        rhs[3 * c + 2] = h2[:, c]
    lhsT[9] = s1h
    lhsT[10] = s1l
    rhs[9] = np.float16(4.0)
    rhs[10] = np.float16(4.0)
    lhsT[11] = np.float16(4.0)
    lhsT[12] = np.float16(4.0)
    rhs[11] = s2h
    rhs[12] = s2l
    return lhsT, rhs


def _register_minmin_reduce():
    """Custom DVE op: out = min(in0, in1); accum_out = min-fold(out).

    Replaces a 5-op fold tree for the per-row min: one pass over the two
    halves of a tile yields the full row min in accum_out. Registered
    dynamically in dve_ops.OPS (the per-NEFF uop table is generated from
    the ops actually used at compile time).
    """
    name = "MINMIN_REDUCE_ANT"
    if name in dve_ops._SUB_OPCODE_FOR_NAME:
        return next(op for op in dve_ops.OPS if op.name == name)

    def _ref(in0, in1, c0, c1, c2):
        out = np.minimum(np.asarray(in0, np.float32), np.asarray(in1, np.float32))
        acc = out.reshape(out.shape[0], -1).min(axis=-1, keepdims=True)
        acc = np.minimum(acc, c0)
        return out, acc

    spec = Spec(body=minn(Src0, Src1), accum=AluOp.MIN, accum_init=C0,
                reference=_ref)
    row = max(dve_ops._SUB_OPCODE_FOR_NAME.values()) + 1
    shas = {}
    for ver in ("v3", "v4"):
        try:
            s = DveOpSpec(name=name, opcode=row, uops=lower(spec, ver=ver),
                          rd1_en=True)
            shas[ver] = s.sha(ver)
        except Exception:
            pass
    op = dve_ops.DveOp(name, spec, subdim=False, uops_sha=shas)
    dve_ops.OPS.append(op)
    dve_ops.CUSTOM_DVE_SPECS[name] = spec
    dve_ops._SUB_OPCODE_FOR_NAME[name] = row
    return op


def build_program(n_rows=4096, n_cols=8192):
    """Build + compile the per-core Bass program (same program on all cores)."""
    ROWT = n_rows // 128  # row tiles
    CG = min(2048, n_cols)  # ACT convert granule (4 PSUM banks)
    NG = n_cols // CG  # granules per row tile
    NMM = CG // 512  # matmuls per granule
    NB = n_cols // 128  # 128-col blocks for the transpose tail
    PER = min(16, NB)  # transpose blocks per PSUM tile

    MINMIN = _register_minmin_reduce()
    nc = bacc.Bacc("TRN2", target_bir_lowering=False, debug=False,
                   num_devices=N_CORES)
    lhsT_d = nc.dram_tensor("lhsT", [K_AUG, n_rows], F16, kind="ExternalInput").ap()
    rhs_d = nc.dram_tensor("rhs", [K_AUG, n_cols], F16, kind="ExternalInput").ap()
    id_d = nc.dram_tensor("ident", [128, 128], F16, kind="ExternalInput").ap()
    out1_d = nc.dram_tensor("out1", [128, ROWT], F32, kind="ExternalOutput").ap()
    out2_d = nc.dram_tensor("out2", [128, NB], F32, kind="ExternalOutput").ap()

    with tile.TileContext(nc) as tc, ExitStack() as ctx:
        const = ctx.enter_context(tc.tile_pool(name="const", bufs=1))
        d16p = ctx.enter_context(tc.tile_pool(name="d16", bufs=3))
        mp = ctx.enter_context(tc.tile_pool(name="m875", bufs=1))
        treep = ctx.enter_context(tc.tile_pool(name="tree", bufs=2))
        outp = ctx.enter_context(tc.tile_pool(name="outs", bufs=1))
        psp = ctx.enter_context(tc.tile_pool(name="ps", bufs=2, space="PSUM"))

        w_sb = const.tile([K_AUG, n_rows], F16)
        nc.sync.dma_start(w_sb[:], lhsT_d)
        r_sb = const.tile([K_AUG, n_cols], F16)
        # chunked so the first matmuls only wait on their own slice; the
        # leading chunks are small to light up the PE->ACT->DVE pipe early
        s = 0
        for w in [512, 512, 1024] + [CG] * (n_cols // CG):
            if s >= n_cols:
                break
            w = min(w, n_cols - s)
            nc.sync.dma_start(r_sb[:, s:s + w], rhs_d[:, s:s + w])
            s += w
        id_sb = const.tile([128, 128], F16)
        nc.sync.dma_start(id_sb[:], id_d)

        M = mp.tile([128, n_cols], F16)
        R = outp.tile([128, ROWT], F32)
        C = outp.tile([128, NB], F32)

        def granule_widths(t):
            # tile 0 leads with small granules so the PE->ACT->DVE pipeline
            # starts as early as possible; steady state uses CG-wide granules
            widths, s = [], 0
            lead = [512, 512, 1024] if t == 0 and n_cols >= 4 * CG else []
            for w in lead + [CG] * (n_cols // 512):
                if s >= n_cols:
                    break
                w = min(w, n_cols - s)
                widths.append(w)
                s += w
            return widths

        for t in range(ROWT):
            d16 = d16p.tile([128, n_cols], F16, tag="d16")
            gs = 0
            for w in granule_widths(t):
                ps = psp.tile([128, w], F32, tag="ps")
                for j in range(w // 512):
                    nc.tensor.matmul(
                        ps[:, 512 * j:512 * (j + 1)],
                        w_sb[:, 128 * t:128 * (t + 1)],
                        r_sb[:, gs + 512 * j:gs + 512 * (j + 1)],
                        start=True, stop=True,
                    )
                nc.scalar.activation(
                    d16[:, gs:gs + w], ps[:],
                    mybir.ActivationFunctionType.Relu,
                )
                # early tiles: granule-wise col-min so DVE consumes each
                # converted granule as it lands during ramp-up
                if t == 0:
                    nc.vector.tensor_copy(M[:, gs:gs + w], d16[:, gs:gs + w])
                elif t <= 2:
                    nc.vector.tensor_tensor(M[:, gs:gs + w], M[:, gs:gs + w],
                                            d16[:, gs:gs + w], op=MIN)
                gs += w
            if t > 2:
                nc.vector.tensor_tensor(M[:], M[:], d16[:], op=MIN)
            # row-min: one fused custom op over the two tile halves
            half = n_cols // 2
            u = treep.tile([128, half], F16, tag="mm_scratch")
            nc.vector._custom_dve(
                MINMIN, out=u[:], in0=d16[:, 0:half], in1=d16[:, half:n_cols],
                s0=65504.0, accum_out=R[:, t:t + 1],
            )

        # column-min of M across its 128 partitions: PE-transpose 128-col
        # blocks into PSUM (manual start/stop: 8 fp16 blocks share a bank),
        # then reduce along the transposed free dim.
        for q in range(NB // PER):
            psT = psp.tile([128, PER * 128], F16, tag="ps")
            for j in range(PER):
                blk = q * PER + j
                nc.tensor.matmul(
                    psT[:, 128 * j:128 * (j + 1)],
                    M[:, 128 * blk:128 * (blk + 1)],
                    id_sb[:],
                    is_transpose=True,
                    start=(j % 8 == 0), stop=(j % 8 == 7),
                )
            nc.vector.tensor_reduce(
                C[:, PER * q:PER * (q + 1)],
                psT[:].rearrange("p (b c) -> p b c", c=128),
                axis=AXX, op=MIN,
            )

        nc.sync.dma_start(out1_d, R[:])
        nc.sync.dma_start(out2_d, C[:])

    nc.compile()
    return nc


_CACHE = {}


def _get_program(n_rows, n_cols):
    key = (n_rows, n_cols)
    if key not in _CACHE:
        _CACHE[key] = build_program(n_rows, n_cols)
    return _CACHE[key]


def run_device(xyz1, xyz2, trace=False):
    """Run the 8-core SPMD program; returns (dist1 [B,N], dist2 [B,M], results)."""
    xyz1 = np.asarray(xyz1)
    xyz2 = np.asarray(xyz2)
    B, N, _ = xyz1.shape
    M = xyz2.shape[1]
    halves = N_CORES // B  # row-halves per batch (2)
    n_rows = N // halves
    nc = _get_program(n_rows, M)

    ident = np.eye(128, dtype=np.float16)
    in_maps = []
    for c in range(N_CORES):
        b, h = divmod(c, halves)
        lhsT, rhs = _prep_core(
            xyz1[b, h * n_rows:(h + 1) * n_rows], xyz2[b])
        in_maps.append({"lhsT": lhsT, "rhs": rhs, "ident": ident})

    res = run_bass_kernel_spmd(nc, in_maps, list(range(N_CORES)), trace=trace)

    dist1 = np.empty((B, N), np.float64)
    dist2p = np.empty((B, halves, M), np.float64)
    for c in range(N_CORES):
        b, h = divmod(c, halves)
        o1 = res.results[c]["out1"].astype(np.float64)  # [128, ROWT]
        o2 = res.results[c]["out2"].astype(np.float64)  # [128, NB]
        dist1[b, h * n_rows:(h + 1) * n_rows] = o1.T.reshape(-1)
        dist2p[b, h] = o2.T.reshape(-1)
    dist1 /= DSCALE
    dist2 = dist2p.min(axis=1) / DSCALE
    return dist1, dist2, res


def _finalize(dist1, dist2):
    cd_p = (np.sqrt(dist1).mean(axis=1) + np.sqrt(dist2).mean(axis=1)) / 2.0
    cd_t = dist1.mean(axis=1) + dist2.mean(axis=1)
    p1 = (dist1 < F1_THRESHOLD).mean(axis=1)
    p2 = (dist2 < F1_THRESHOLD).mean(axis=1)
    denom = p1 + p2
    f1 = np.where(denom > 0, 2.0 * p1 * p2 / np.where(denom > 0, denom, 1.0), 0.0)
    return (cd_p.astype(np.float32), cd_t.astype(np.float32),
            f1.astype(np.float32))


def kernel(xyz1, xyz2):
    dist1, dist2, _ = run_device(xyz1, xyz2, trace=False)
    return _finalize(dist1, dist2)
